# revision 34
# baseline (speedup 1.0000x reference)
"""Trainium2 Bass kernel for nn_DeltaModel (scatter_memory).

Algorithm: every per-token quantity (embedding -> MLP -> LayerNorm -> k/v/q
projections) is a pure function of the vocab id (V=64), so the encode collapses
to 64-row tables computed once on the host (pure weight preprocessing).  The
delta-rule scan
    M_{t+1} = M_t + (v_t - M_t k_t) k_t^T ,  out = M_T q
collapses (since only M_T @ q is needed) to a backward vector recursion
    u <- q;  for t = T-1..0:  a_t = k_t . u ;  u <- u - a_t k_t
    M_T q = sum_t a_t v_t

Gauge trick: store the state in the "key gauge" X = u / k_cur (elementwise).
Both halves of a step then fit AFFINE_MUL_REDUCE (out=(in0*s0+s1)*in1,
accum=sum(out)), whose semaphore update rides the accumulator-read aux
instruction and therefore chains ~60ns/op faster than scalar_tensor_tensor:
    dot:    accum = sum(X * (-k^2))            = -a_t
    update: X'    = (X + (-a_t)) * (k_t/k_nxt)
The -1 on the alphas is folded into a negated v-table.  Per step both streams
come from one 512B row of a 4096-row pair table [-k_a^2 | k_a/k_b], indirect
DMA row-gathered by pair id; the answer sum runs as per-chunk PE matmuls
accumulated in a persistent PSUM bank.

Sharding: pure data parallel, batch 256 -> 8 cores x 32.
"""

import numpy as np

B, L, V, H = 256, 2048, 64, 64  # problem shape (hardcoded per spec)
NCORES = 8
BL = B // NCORES  # 32
T_FULL = L - 1  # 2047
SUPER = 128  # sweep gather tile (time steps)
CHUNK = 128  # answer-matmul chunk (time steps)

_CACHE = {}
LAST_RESULTS = None


def _build_nc(T):
    import concourse.bass as bass
    import concourse.mybir as mybir
    import concourse.tile as tile
    from concourse import bacc

    f32 = mybir.dt.float32
    i16 = mybir.dt.int16
    Alu = mybir.AluOpType

    nc = bacc.Bacc("TRN2", target_bir_lowering=False, debug=False,
                   num_devices=NCORES)

    # ---- I/O -----------------------------------------------------------
    TP = (T + SUPER - 1) // SUPER * SUPER  # padded step count (2048)
    NST = TP // SUPER
    NCH = TP // CHUNK
    kidx_d = nc.dram_tensor("kidx", [128, NST * SUPER * 8], i16,
                            kind="ExternalInput")
    vidx_d = nc.dram_tensor("vidx", [128, NCH * BL * CHUNK // 16], i16,
                            kind="ExternalInput")
    qrtab_d = nc.dram_tensor("qrtab", [V * V, 2 * H], f32,
                             kind="ExternalInput")
    vtn_d = nc.dram_tensor("vtn", [V, H], f32, kind="ExternalInput")
    x0_d = nc.dram_tensor("x0", [BL, H], f32, kind="ExternalInput")
    wrpb_d = nc.dram_tensor("wrpb", [H + 1, H], f32, kind="ExternalInput")
    wrpbias_d = nc.dram_tensor("wrpbias", [1, H], f32, kind="ExternalInput")
    woutb_d = nc.dram_tensor("woutb", [H + 1, V], f32, kind="ExternalInput")
    woutbias_d = nc.dram_tensor("woutbias", [1, V], f32, kind="ExternalInput")
    iden_d = nc.dram_tensor("iden", [128, 128], f32, kind="ExternalInput")
    out_d = nc.dram_tensor("out", [V, BL], f32, kind="ExternalOutput")

    with tile.TileContext(nc) as tc:
        with (
            tc.tile_pool(name="const", bufs=1) as cp,
            tc.tile_pool(name="setup", bufs=1) as sp,
            tc.tile_pool(name="setup_ps", bufs=2, space="PSUM") as spp,
            tc.tile_pool(name="sweep", bufs=1) as swp,
            tc.tile_pool(name="qst_p", bufs=2) as qp,
            tc.tile_pool(name="vst", bufs=3) as vp,
            tc.tile_pool(name="ans_ps", bufs=1, space="PSUM") as ap_pool,
            tc.tile_pool(name="at_ps", bufs=2, space="PSUM") as atp,
        ):
            # ---- load constants ---------------------------------------
            def load(pool, dram, shape, tag, dtype=f32):
                t = pool.tile(shape, dtype, tag=tag, name=tag)
                nc.gpsimd.dma_start(out=t[:], in_=dram.ap())
                return t

            x0 = load(cp, x0_d, [BL, H], "c_x0")

            # ---- main sweep -------------------------------------------
            X = swp.tile([BL, H], f32, name="X")
            nc.vector.tensor_copy(X[:], x0[:])
            junk = swp.tile([BL, H], f32, name="junk")
            junkacc = swp.tile([BL, 1], f32, name="junkacc")
            alpha = swp.tile([BL, TP], f32, name="alpha")
            # only the padded tail column is read without being written
            nc.vector.memset(alpha[:, T:TP], 0.0)
            # one PSUM bank parks all 16x32 per-(chunk,b) answer columns,
            # b-major so the final reduce is a contiguous [H, BL, NCH] view
            cps_all = ap_pool.tile([H, BL, NCH], f32, name="cps_all")

            qtiles = {}
            vtiles = {}

            def issue_gathers(st, first=False):
                kix = qp.tile([128, SUPER * 8], i16, tag="kix", name="kix")
                if first:
                    # tiny first slice so the first gather can start early
                    nc.gpsimd.dma_start(
                        out=kix[:, 0:16],
                        in_=kidx_d.ap()[:, st * SUPER * 8:
                                        st * SUPER * 8 + 16])
                    nc.gpsimd.dma_start(
                        out=kix[:, 16:SUPER * 8],
                        in_=kidx_d.ap()[:, st * SUPER * 8 + 16:
                                        (st + 1) * SUPER * 8])
                else:
                    nc.gpsimd.dma_start(
                        out=kix[:], in_=kidx_d.ap()[:, st * SUPER * 8:
                                                    (st + 1) * SUPER * 8])
                q = qp.tile([128, SUPER, 2 * H], f32, tag="qst", name="qst")
                # HW SWDGE caps one gather at ~1024 idxs.  For the very
                # first tile, fetch the first 2 step-slots separately so the
                # chain can start ~1us earlier.
                if first:
                    nc.gpsimd.dma_gather(
                        out_ap=q[:, 0:2, :], in_ap=qrtab_d.ap(),
                        idxs_ap=kix[:, 0:16],
                        num_idxs=256, num_idxs_reg=256, elem_size=2 * H)
                    nc.gpsimd.dma_gather(
                        out_ap=q[:, 2:8, :], in_ap=qrtab_d.ap(),
                        idxs_ap=kix[:, 16:64],
                        num_idxs=768, num_idxs_reg=768, elem_size=2 * H)
                    pieces = range(1, SUPER * 128 // 1024)
                else:
                    pieces = range(SUPER * 128 // 1024)
                npc = SUPER * 128 // 1024
                for piece in pieces:
                    sl = SUPER // npc
                    nc.gpsimd.dma_gather(
                        out_ap=q[:, piece * sl:(piece + 1) * sl, :],
                        in_ap=qrtab_d.ap(),
                        idxs_ap=kix[:, piece * 64:(piece + 1) * 64],
                        num_idxs=1024, num_idxs_reg=1024, elem_size=2 * H)
                qtiles[st] = q

            def issue_vgathers(ci):
                vst = vp.tile([CHUNK, BL, H], f32, tag="vst", name="vst")
                vbase = ci * BL * CHUNK // 16
                for piece in range(BL * CHUNK // 1024):
                    nc.gpsimd.dma_gather(
                        out_ap=vst[:, piece * 8:(piece + 1) * 8, :],
                        in_ap=vtn_d.ap(),
                        idxs_ap=vidx_sb[:, vbase + piece * 64:
                                        vbase + (piece + 1) * 64],
                        num_idxs=1024, num_idxs_reg=1024, elem_size=H)
                vtiles[ci] = vst

            for st in range(min(1, NST)):
                issue_gathers(st, first=True)
            # bulk constants are not needed until the first answer chunk /
            # epilogue; load them after the first gathers are in flight
            vidx_sb = load(cp, vidx_d, [128, NCH * BL * CHUNK // 16],
                           "c_vidx", i16)
            wrpb = load(cp, wrpb_d, [H + 1, H], "c_wrpb")
            wrpbias = load(cp, wrpbias_d, [1, H], "c_wrpbias")
            woutb = load(cp, woutb_d, [H + 1, V], "c_woutb")
            woutbias = load(cp, woutbias_d, [1, V], "c_woutbias")
            iden = load(cp, iden_d, [128, 128], "c_iden")
            issue_vgathers(0)
            for st in range(NST):
                if st + 1 < NST:
                    issue_gathers(st + 1)
                    issue_vgathers(st + 1)
                q = qtiles.pop(st)
                t0 = st * SUPER
                sc = min(SUPER, T - t0)
                for j in range(sc):
                    tau = t0 + j
                    # accum = sum(X * (-k^2)) = -a_tau ; out is junk
                    nc.vector.affine_mul_reduce(
                        out=junk[:], accum_out=alpha[:, tau:tau + 1],
                        in0=X[:], in1=q[:BL, j, 0:H], scale=1.0, bias=0.0)
                    # X' = (X + (-a_tau)) * (k_tau / k_next)
                    nc.vector.affine_mul_reduce(
                        out=X[:], accum_out=junkacc[:],
                        in0=X[:], in1=q[:BL, j, H:2 * H], scale=1.0,
                        bias=alpha[:, tau:tau + 1])
                # answer chunks of this supertile (full CHUNK frames; alpha
                # is zero-padded past T so junk v rows contribute nothing)
                for c0 in range(0, SUPER, CHUNK):
                    tau0 = t0 + c0
                    ci = tau0 // CHUNK
                    vst = vtiles.pop(ci)
                    at_ps = atp.tile([CHUNK, BL], f32, name="at_ps")
                    nc.tensor.transpose(at_ps[:],
                                        alpha[:, tau0:tau0 + CHUNK],
                                        iden[:BL, :BL])
                    atb = vp.tile([CHUNK, BL], f32, tag="atb", name="atb")
                    nc.scalar.copy(atb[:], at_ps[:])
                    for b in range(BL):
                        nc.tensor.matmul(cps_all[:, b, ci:ci + 1],
                                         lhsT=vst[:, b, :],
                                         rhs=atb[:, b:b + 1],
                                         start=True, stop=True)

            # ---- epilogue (output stays [V, BL]; host transposes) -----
            ones = sp.tile([1, BL], f32, name="ones")
            nc.vector.memset(ones[:], 1.0)
            ans_acc = sp.tile([H, BL], f32, name="ans_acc")
            nc.vector.tensor_reduce(
                ans_acc[:], cps_all[:, :, :],
                axis=mybir.AxisListType.X, op=Alu.add)
            rps = spp.tile([H, BL], f32, tag="sps", name="rps")
            nc.tensor.matmul(rps[:], lhsT=wrpbias[:], rhs=ones[:],
                             start=True, stop=False)
            nc.tensor.matmul(rps[:], lhsT=wrpb[:H, :], rhs=ans_acc[:],
                             start=False, stop=True)
            rx = sp.tile([H, BL], f32, name="rx")
            nc.scalar.copy(rx[:], rps[:])
            ops_ = spp.tile([V, BL], f32, tag="sps", name="ops_")
            nc.tensor.matmul(ops_[:], lhsT=woutbias[:], rhs=ones[:],
                             start=True, stop=False)
            nc.tensor.matmul(ops_[:], lhsT=woutb[:H, :], rhs=rx[:],
                             start=False, stop=True)
            o_sb = sp.tile([V, BL], f32, name="o_sb")
            nc.scalar.copy(o_sb[:], ops_[:])
            nc.gpsimd.dma_start(out=out_d.ap(), in_=o_sb[:])

    nc.compile()
    return nc


def _tables(inputs):
    """Host-side weight preprocessing: collapse the token-wise encode to
    64-row tables (pure function of the weights), mirroring reference.py."""
    f = np.float32
    embed = np.asarray(inputs["embed"], f)
    W1 = np.asarray(inputs["W1"], f)
    b1 = np.asarray(inputs["b1"], f)
    W2 = np.asarray(inputs["W2"], f)
    b2 = np.asarray(inputs["b2"], f)
    gamma = np.asarray(inputs["gamma"], f)
    beta = np.asarray(inputs["beta"], f)
    Wk = np.asarray(inputs["Wk"], f)
    Wv = np.asarray(inputs["Wv"], f)
    Wq = np.asarray(inputs["Wq"], f)

    e = embed  # [V, H]
    ff = np.maximum(e @ W1.T + b1, 0.0) @ W2.T + b2
    h = e + ff
    mu = h.mean(-1, keepdims=True)
    var = ((h - mu) ** 2).mean(-1, keepdims=True)
    hs = (h - mu) / np.sqrt(var + 1e-5) * gamma + beta
    k = hs @ Wk.T
    kn = k / np.maximum(np.linalg.norm(k, axis=-1, keepdims=True), 1e-12)
    vt = hs @ Wv.T
    qt = hs @ Wq.T
    return kn.astype(f), vt.astype(f), qt.astype(f)


def _marshal(inputs, T):
    f = np.float32
    seq = np.asarray(inputs["seq"])
    Wrp = np.asarray(inputs["Wrp"], f)
    brp = np.asarray(inputs["brp"], f)
    Wout = np.asarray(inputs["Wout"], f)
    bout = np.asarray(inputs["bout"], f)

    kn, vt, qt = _tables(inputs)
    # guard: the gauge divides by k components; clamp exact/near zeros
    # (|k| ~ 1e-12 changes the math by O(1e-24) but keeps ratios finite)
    kn = np.where(np.abs(kn) < 1e-12, 1e-12, kn).astype(f)

    # combined row for pair (a,b): [-k_a^2 | k_a / k_b]
    qrtab = np.empty((V, V, 2 * H), f)
    qrtab[:, :, :H] = (-kn * kn)[:, None, :]
    qrtab[:, :, H:] = kn[:, None, :] / kn[None, :, :]
    vtn = (-vt).astype(f)

    shared = {
        "qrtab": qrtab.reshape(V * V, 2 * H),
        "vtn": vtn,
        "wrpb": np.vstack([Wrp.T, brp[None]]).astype(f),
        "wrpbias": brp[None].astype(f),
        "woutb": np.vstack([Wout.T, bout[None]]).astype(f),
        "woutbias": bout[None].astype(f),
        "iden": np.eye(128, dtype=f),
    }
    TP = (T + SUPER - 1) // SUPER * SUPER
    NST = TP // SUPER
    NCH = TP // CHUNK

    def wrap(flat):
        n = flat.size
        w16 = np.ascontiguousarray(flat.reshape(n // 16, 16).T).astype(np.int16)
        return np.tile(w16, (8, 1))

    in_maps = []
    for c in range(NCORES):
        sl = slice(c * BL, (c + 1) * BL)
        sseq = seq[sl]
        # reversed-time ids: ids[b, tau] = seq[b, (T-1) - tau]
        ids = np.ascontiguousarray(sseq[:, T - 1::-1]).astype(np.int64)
        idsp = np.zeros((BL, TP), np.int64)
        idsp[:, :T] = ids
        # pair ids: cur*64 + next (next in tau order); tail pairs with self
        nxt = np.zeros((BL, TP), np.int64)
        nxt[:, :T - 1] = ids[:, 1:]
        nxt[:, T - 1:] = ids[:, T - 1:T]
        pair = idsp * 64 + nxt
        pair[:, T:] = 0
        # qr-stream: i = slot*128 + p ; p<BL -> pair[p, t0+slot], else 0
        kblocks = []
        for st in range(NST):
            blk = np.zeros((SUPER, 128), np.int64)
            blk[:, :BL] = pair[:, st * SUPER:(st + 1) * SUPER].T
            kblocks.append(wrap(blk.reshape(-1)))
        # v-stream: i = b*128 + tau ; chunk frames of CHUNK
        vblocks = []
        for ci in range(NCH):
            blk = idsp[:, ci * CHUNK:(ci + 1) * CHUNK]  # [BL, CHUNK]
            vblocks.append(wrap(blk.reshape(-1)))
        # X0 = q_row / k(first step) ; q uses the LAST token id seq[:, L-1]
        qrows = qt[sseq[:, L - 1]]
        krows = kn[ids[:, 0]]
        m = dict(shared)
        m["x0"] = (qrows / krows).astype(f)
        m["kidx"] = np.concatenate(kblocks, axis=1)
        m["vidx"] = np.concatenate(vblocks, axis=1)
        in_maps.append(m)
    return in_maps


def kernel(**inputs):
    global LAST_RESULTS
    import os
    from concourse.bass_utils import run_bass_kernel_spmd

    T = T_FULL
    if "nc" not in _CACHE:
        _CACHE["nc"] = _build_nc(T)
    nc = _CACHE["nc"]
    in_maps = _marshal(inputs, T)
    trace = bool(int(os.environ.get("KERNEL_TRACE", "0")))
    res = run_bass_kernel_spmd(nc, in_maps, core_ids=list(range(NCORES)),
                               trace=trace)
    LAST_RESULTS = res
    out = np.concatenate([res.results[c]["out"].T for c in range(NCORES)],
                         axis=0)
    return out.astype(np.float32)


# revision 37
# speedup vs baseline: 1.0011x; 1.0011x over previous
"""Trainium2 Bass kernel for nn_DeltaModel (scatter_memory).

Algorithm: every per-token quantity (embedding -> MLP -> LayerNorm -> k/v/q
projections) is a pure function of the vocab id (V=64), so the encode collapses
to 64-row tables computed once on the host (pure weight preprocessing).  The
delta-rule scan
    M_{t+1} = M_t + (v_t - M_t k_t) k_t^T ,  out = M_T q
collapses (since only M_T @ q is needed) to a backward vector recursion
    u <- q;  for t = T-1..0:  a_t = k_t . u ;  u <- u - a_t k_t
    M_T q = sum_t a_t v_t

Gauge trick: store the state in the "key gauge" X = u / k_cur (elementwise).
Both halves of a step then fit AFFINE_MUL_REDUCE (out=(in0*s0+s1)*in1,
accum=sum(out)), whose semaphore update rides the accumulator-read aux
instruction and therefore chains ~60ns/op faster than scalar_tensor_tensor:
    dot:    accum = sum(X * (-k^2))            = -a_t
    update: X'    = (X + (-a_t)) * (k_t/k_nxt)
The -1 on the alphas is folded into a negated v-table.  Per step both streams
come from one 512B row of a 4096-row pair table [-k_a^2 | k_a/k_b], indirect
DMA row-gathered by pair id; the answer sum runs as per-chunk PE matmuls
accumulated in a persistent PSUM bank.

Sharding: pure data parallel, batch 256 -> 8 cores x 32.
"""

import numpy as np

B, L, V, H = 256, 2048, 64, 64  # problem shape (hardcoded per spec)
NCORES = 8
BL = B // NCORES  # 32
T_FULL = L - 1  # 2047
SUPER = 128  # sweep gather tile (time steps)
CHUNK = 128  # answer-matmul chunk (time steps)

_CACHE = {}
LAST_RESULTS = None


def _build_nc(T):
    import concourse.bass as bass
    import concourse.mybir as mybir
    import concourse.tile as tile
    from concourse import bacc

    f32 = mybir.dt.float32
    i16 = mybir.dt.int16
    Alu = mybir.AluOpType

    nc = bacc.Bacc("TRN2", target_bir_lowering=False, debug=False,
                   num_devices=NCORES)

    # ---- I/O -----------------------------------------------------------
    TP = (T + SUPER - 1) // SUPER * SUPER  # padded step count (2048)
    NST = TP // SUPER
    NCH = TP // CHUNK
    kidx_d = nc.dram_tensor("kidx", [128, NST * SUPER * 8], i16,
                            kind="ExternalInput")
    vidx_d = nc.dram_tensor("vidx", [128, NCH * BL * CHUNK // 16], i16,
                            kind="ExternalInput")
    qrtab_d = nc.dram_tensor("qrtab", [V * V, 2 * H], f32,
                             kind="ExternalInput")
    vtn_d = nc.dram_tensor("vtn", [V, H], f32, kind="ExternalInput")
    x0_d = nc.dram_tensor("x0", [BL, H], f32, kind="ExternalInput")
    wrpb_d = nc.dram_tensor("wrpb", [H + 1, H], f32, kind="ExternalInput")
    wrpbias_d = nc.dram_tensor("wrpbias", [1, H], f32, kind="ExternalInput")
    woutb_d = nc.dram_tensor("woutb", [H + 1, V], f32, kind="ExternalInput")
    woutbias_d = nc.dram_tensor("woutbias", [1, V], f32, kind="ExternalInput")
    iden_d = nc.dram_tensor("iden", [128, 128], f32, kind="ExternalInput")
    out_d = nc.dram_tensor("out", [V, BL], f32, kind="ExternalOutput")

    with tile.TileContext(nc) as tc:
        with (
            tc.tile_pool(name="const", bufs=1) as cp,
            tc.tile_pool(name="setup", bufs=1) as sp,
            tc.tile_pool(name="setup_ps", bufs=2, space="PSUM") as spp,
            tc.tile_pool(name="sweep", bufs=1) as swp,
            tc.tile_pool(name="qst_p", bufs=2) as qp,
            tc.tile_pool(name="vst", bufs=3) as vp,
            tc.tile_pool(name="ans_ps", bufs=1, space="PSUM") as ap_pool,
            tc.tile_pool(name="at_ps", bufs=2, space="PSUM") as atp,
        ):
            # ---- load constants ---------------------------------------
            def load(pool, dram, shape, tag, dtype=f32):
                t = pool.tile(shape, dtype, tag=tag, name=tag)
                nc.gpsimd.dma_start(out=t[:], in_=dram.ap())
                return t

            x0 = load(cp, x0_d, [BL, H], "c_x0")

            # ---- main sweep -------------------------------------------
            X = swp.tile([BL, H], f32, name="X")
            junk = swp.tile([BL, H], f32, name="junk")
            junkacc = swp.tile([BL, 1], f32, name="junkacc")
            alpha = swp.tile([BL, TP], f32, name="alpha")
            # only the padded tail column is read without being written
            nc.vector.memset(alpha[:, T:TP], 0.0)
            # one PSUM bank parks all 16x32 per-(chunk,b) answer columns,
            # b-major so the final reduce is a contiguous [H, BL, NCH] view
            cps_all = ap_pool.tile([H, BL, NCH], f32, name="cps_all")

            qtiles = {}
            vtiles = {}

            def issue_gathers(st, first=False):
                kix = qp.tile([128, SUPER * 8], i16, tag="kix", name="kix")
                nc.gpsimd.dma_start(
                    out=kix[:], in_=kidx_d.ap()[:, st * SUPER * 8:
                                                (st + 1) * SUPER * 8])
                q = qp.tile([128, SUPER, 2 * H], f32, tag="qst", name="qst")
                # HW SWDGE caps one gather at ~1024 idxs.  For the very
                # first tile, fetch the first 2 step-slots separately so the
                # chain can start ~1us earlier.
                if first:
                    nc.gpsimd.dma_gather(
                        out_ap=q[:, 0:2, :], in_ap=qrtab_d.ap(),
                        idxs_ap=kix[:, 0:16],
                        num_idxs=256, num_idxs_reg=256, elem_size=2 * H)
                    nc.gpsimd.dma_gather(
                        out_ap=q[:, 2:8, :], in_ap=qrtab_d.ap(),
                        idxs_ap=kix[:, 16:64],
                        num_idxs=768, num_idxs_reg=768, elem_size=2 * H)
                    pieces = range(1, SUPER * 128 // 1024)
                else:
                    pieces = range(SUPER * 128 // 1024)
                npc = SUPER * 128 // 1024
                for piece in pieces:
                    sl = SUPER // npc
                    nc.gpsimd.dma_gather(
                        out_ap=q[:, piece * sl:(piece + 1) * sl, :],
                        in_ap=qrtab_d.ap(),
                        idxs_ap=kix[:, piece * 64:(piece + 1) * 64],
                        num_idxs=1024, num_idxs_reg=1024, elem_size=2 * H)
                qtiles[st] = q

            def issue_vgathers(ci):
                vst = vp.tile([CHUNK, BL, H], f32, tag="vst", name="vst")
                vbase = ci * BL * CHUNK // 16
                for piece in range(BL * CHUNK // 1024):
                    nc.gpsimd.dma_gather(
                        out_ap=vst[:, piece * 8:(piece + 1) * 8, :],
                        in_ap=vtn_d.ap(),
                        idxs_ap=vidx_sb[:, vbase + piece * 64:
                                        vbase + (piece + 1) * 64],
                        num_idxs=1024, num_idxs_reg=1024, elem_size=H)
                vtiles[ci] = vst

            for st in range(min(1, NST)):
                issue_gathers(st, first=True)
            # bulk constants are not needed until the first answer chunk /
            # epilogue; load them after the first gathers are in flight
            vidx_sb = load(cp, vidx_d, [128, NCH * BL * CHUNK // 16],
                           "c_vidx", i16)
            wrpb = load(cp, wrpb_d, [H + 1, H], "c_wrpb")
            wrpbias = load(cp, wrpbias_d, [1, H], "c_wrpbias")
            woutb = load(cp, woutb_d, [H + 1, V], "c_woutb")
            woutbias = load(cp, woutbias_d, [1, V], "c_woutbias")
            iden = load(cp, iden_d, [128, 128], "c_iden")
            issue_vgathers(0)
            for st in range(NST):
                if st + 1 < NST:
                    issue_gathers(st + 1)
                    issue_vgathers(st + 1)
                q = qtiles.pop(st)
                t0 = st * SUPER
                sc = min(SUPER, T - t0)
                for j in range(sc):
                    tau = t0 + j
                    xin = x0 if tau == 0 else X
                    # accum = sum(X * (-k^2)) = -a_tau ; out is junk
                    nc.vector.affine_mul_reduce(
                        out=junk[:], accum_out=alpha[:, tau:tau + 1],
                        in0=xin[:], in1=q[:BL, j, 0:H], scale=1.0, bias=0.0)
                    # X' = (X + (-a_tau)) * (k_tau / k_next)
                    nc.vector.affine_mul_reduce(
                        out=X[:], accum_out=junkacc[:],
                        in0=xin[:], in1=q[:BL, j, H:2 * H], scale=1.0,
                        bias=alpha[:, tau:tau + 1])
                # answer chunks of this supertile (full CHUNK frames; alpha
                # is zero-padded past T so junk v rows contribute nothing)
                for c0 in range(0, SUPER, CHUNK):
                    tau0 = t0 + c0
                    ci = tau0 // CHUNK
                    vst = vtiles.pop(ci)
                    at_ps = atp.tile([CHUNK, BL], f32, name="at_ps")
                    nc.tensor.transpose(at_ps[:],
                                        alpha[:, tau0:tau0 + CHUNK],
                                        iden[:BL, :BL])
                    atb = vp.tile([CHUNK, BL], f32, tag="atb", name="atb")
                    nc.scalar.copy(atb[:], at_ps[:])
                    for b in range(BL):
                        nc.tensor.matmul(cps_all[:, b, ci:ci + 1],
                                         lhsT=vst[:, b, :],
                                         rhs=atb[:, b:b + 1],
                                         start=True, stop=True)

            # ---- epilogue (output stays [V, BL]; host transposes) -----
            ones = sp.tile([1, BL], f32, name="ones")
            nc.vector.memset(ones[:], 1.0)
            ans_acc = sp.tile([H, BL], f32, name="ans_acc")
            nc.vector.tensor_reduce(
                ans_acc[:], cps_all[:, :, :],
                axis=mybir.AxisListType.X, op=Alu.add)
            rps = spp.tile([H, BL], f32, tag="sps", name="rps")
            nc.tensor.matmul(rps[:], lhsT=wrpbias[:], rhs=ones[:],
                             start=True, stop=False)
            nc.tensor.matmul(rps[:], lhsT=wrpb[:H, :], rhs=ans_acc[:],
                             start=False, stop=True)
            rx = sp.tile([H, BL], f32, name="rx")
            nc.scalar.copy(rx[:], rps[:])
            ops_ = spp.tile([V, BL], f32, tag="sps", name="ops_")
            nc.tensor.matmul(ops_[:], lhsT=woutbias[:], rhs=ones[:],
                             start=True, stop=False)
            nc.tensor.matmul(ops_[:], lhsT=woutb[:H, :], rhs=rx[:],
                             start=False, stop=True)
            o_sb = sp.tile([V, BL], f32, name="o_sb")
            nc.scalar.copy(o_sb[:], ops_[:])
            nc.gpsimd.dma_start(out=out_d.ap(), in_=o_sb[:])

    nc.compile()
    return nc


def _tables(inputs):
    """Host-side weight preprocessing: collapse the token-wise encode to
    64-row tables (pure function of the weights), mirroring reference.py."""
    f = np.float32
    embed = np.asarray(inputs["embed"], f)
    W1 = np.asarray(inputs["W1"], f)
    b1 = np.asarray(inputs["b1"], f)
    W2 = np.asarray(inputs["W2"], f)
    b2 = np.asarray(inputs["b2"], f)
    gamma = np.asarray(inputs["gamma"], f)
    beta = np.asarray(inputs["beta"], f)
    Wk = np.asarray(inputs["Wk"], f)
    Wv = np.asarray(inputs["Wv"], f)
    Wq = np.asarray(inputs["Wq"], f)

    e = embed  # [V, H]
    ff = np.maximum(e @ W1.T + b1, 0.0) @ W2.T + b2
    h = e + ff
    mu = h.mean(-1, keepdims=True)
    var = ((h - mu) ** 2).mean(-1, keepdims=True)
    hs = (h - mu) / np.sqrt(var + 1e-5) * gamma + beta
    k = hs @ Wk.T
    kn = k / np.maximum(np.linalg.norm(k, axis=-1, keepdims=True), 1e-12)
    vt = hs @ Wv.T
    qt = hs @ Wq.T
    return kn.astype(f), vt.astype(f), qt.astype(f)


def _marshal(inputs, T):
    f = np.float32
    seq = np.asarray(inputs["seq"])
    Wrp = np.asarray(inputs["Wrp"], f)
    brp = np.asarray(inputs["brp"], f)
    Wout = np.asarray(inputs["Wout"], f)
    bout = np.asarray(inputs["bout"], f)

    kn, vt, qt = _tables(inputs)
    # guard: the gauge divides by k components; clamp exact/near zeros
    # (|k| ~ 1e-12 changes the math by O(1e-24) but keeps ratios finite)
    kn = np.where(np.abs(kn) < 1e-12, 1e-12, kn).astype(f)

    # combined row for pair (a,b): [-k_a^2 | k_a / k_b]
    qrtab = np.empty((V, V, 2 * H), f)
    qrtab[:, :, :H] = (-kn * kn)[:, None, :]
    qrtab[:, :, H:] = kn[:, None, :] / kn[None, :, :]
    vtn = (-vt).astype(f)

    shared = {
        "qrtab": qrtab.reshape(V * V, 2 * H),
        "vtn": vtn,
        "wrpb": np.vstack([Wrp.T, brp[None]]).astype(f),
        "wrpbias": brp[None].astype(f),
        "woutb": np.vstack([Wout.T, bout[None]]).astype(f),
        "woutbias": bout[None].astype(f),
        "iden": np.eye(128, dtype=f),
    }
    TP = (T + SUPER - 1) // SUPER * SUPER
    NST = TP // SUPER
    NCH = TP // CHUNK

    def wrap(flat):
        n = flat.size
        w16 = np.ascontiguousarray(flat.reshape(n // 16, 16).T).astype(np.int16)
        return np.tile(w16, (8, 1))

    in_maps = []
    for c in range(NCORES):
        sl = slice(c * BL, (c + 1) * BL)
        sseq = seq[sl]
        # reversed-time ids: ids[b, tau] = seq[b, (T-1) - tau]
        ids = np.ascontiguousarray(sseq[:, T - 1::-1]).astype(np.int64)
        idsp = np.zeros((BL, TP), np.int64)
        idsp[:, :T] = ids
        # pair ids: cur*64 + next (next in tau order); tail pairs with self
        nxt = np.zeros((BL, TP), np.int64)
        nxt[:, :T - 1] = ids[:, 1:]
        nxt[:, T - 1:] = ids[:, T - 1:T]
        pair = idsp * 64 + nxt
        pair[:, T:] = 0
        # qr-stream: i = slot*128 + p ; p<BL -> pair[p, t0+slot], else 0
        kblocks = []
        for st in range(NST):
            blk = np.zeros((SUPER, 128), np.int64)
            blk[:, :BL] = pair[:, st * SUPER:(st + 1) * SUPER].T
            kblocks.append(wrap(blk.reshape(-1)))
        # v-stream: i = b*128 + tau ; chunk frames of CHUNK
        vblocks = []
        for ci in range(NCH):
            blk = idsp[:, ci * CHUNK:(ci + 1) * CHUNK]  # [BL, CHUNK]
            vblocks.append(wrap(blk.reshape(-1)))
        # X0 = q_row / k(first step) ; q uses the LAST token id seq[:, L-1]
        qrows = qt[sseq[:, L - 1]]
        krows = kn[ids[:, 0]]
        m = dict(shared)
        m["x0"] = (qrows / krows).astype(f)
        m["kidx"] = np.concatenate(kblocks, axis=1)
        m["vidx"] = np.concatenate(vblocks, axis=1)
        in_maps.append(m)
    return in_maps


def kernel(**inputs):
    global LAST_RESULTS
    import os
    from concourse.bass_utils import run_bass_kernel_spmd

    T = T_FULL
    if "nc" not in _CACHE:
        _CACHE["nc"] = _build_nc(T)
    nc = _CACHE["nc"]
    in_maps = _marshal(inputs, T)
    trace = bool(int(os.environ.get("KERNEL_TRACE", "0")))
    res = run_bass_kernel_spmd(nc, in_maps, core_ids=list(range(NCORES)),
                               trace=trace)
    LAST_RESULTS = res
    out = np.concatenate([res.results[c]["out"].T for c in range(NCORES)],
                         axis=0)
    return out.astype(np.float32)


# revision 42
# speedup vs baseline: 1.0031x; 1.0020x over previous
"""Trainium2 Bass kernel for nn_DeltaModel (scatter_memory).

Algorithm: every per-token quantity (embedding -> MLP -> LayerNorm -> k/v/q
projections) is a pure function of the vocab id (V=64), so the encode collapses
to 64-row tables computed once on the host (pure weight preprocessing).  The
delta-rule scan
    M_{t+1} = M_t + (v_t - M_t k_t) k_t^T ,  out = M_T q
collapses (since only M_T @ q is needed) to a backward vector recursion
    u <- q;  for t = T-1..0:  a_t = k_t . u ;  u <- u - a_t k_t
    M_T q = sum_t a_t v_t

Gauge trick: store the state in the "key gauge" X = u / k_cur (elementwise).
Both halves of a step then fit AFFINE_MUL_REDUCE (out=(in0*s0+s1)*in1,
accum=sum(out)), whose semaphore update rides the accumulator-read aux
instruction and therefore chains ~60ns/op faster than scalar_tensor_tensor:
    dot:    accum = sum(X * (-k^2))            = -a_t
    update: X'    = (X + (-a_t)) * (k_t/k_nxt)
The -1 on the alphas is folded into a negated v-table.  Per step both streams
come from one 512B row of a 4096-row pair table [-k_a^2 | k_a/k_b], indirect
DMA row-gathered by pair id; the answer sum runs as per-chunk PE matmuls
accumulated in a persistent PSUM bank.

Sharding: pure data parallel, batch 256 -> 8 cores x 32.
"""

import numpy as np

B, L, V, H = 256, 2048, 64, 64  # problem shape (hardcoded per spec)
NCORES = 8
BL = B // NCORES  # 32
T_FULL = L - 1  # 2047
SUPER = 128  # sweep gather tile (time steps)
CHUNK = 128  # answer-matmul chunk (time steps)

_CACHE = {}
LAST_RESULTS = None


def _build_nc(T):
    import concourse.bass as bass
    import concourse.mybir as mybir
    import concourse.tile as tile
    from concourse import bacc

    f32 = mybir.dt.float32
    i16 = mybir.dt.int16
    Alu = mybir.AluOpType

    nc = bacc.Bacc("TRN2", target_bir_lowering=False, debug=False,
                   num_devices=NCORES)

    # ---- I/O -----------------------------------------------------------
    TP = (T + SUPER - 1) // SUPER * SUPER  # padded step count (2048)
    NST = TP // SUPER
    NCH = TP // CHUNK
    kidx_d = nc.dram_tensor("kidx", [128, NST * SUPER * 8], i16,
                            kind="ExternalInput")
    vidx_d = nc.dram_tensor("vidx", [128, NCH * BL * CHUNK // 16], i16,
                            kind="ExternalInput")
    qrtab_d = nc.dram_tensor("qrtab", [V * V, 2 * H], f32,
                             kind="ExternalInput")
    vtn_d = nc.dram_tensor("vtn", [V, H], f32, kind="ExternalInput")
    x0_d = nc.dram_tensor("x0", [BL, H], f32, kind="ExternalInput")
    qr0_d = nc.dram_tensor("qr0", [BL, 8 * 2 * H], f32, kind="ExternalInput")
    wrpb_d = nc.dram_tensor("wrpb", [H + 1, H], f32, kind="ExternalInput")
    wrpbias_d = nc.dram_tensor("wrpbias", [1, H], f32, kind="ExternalInput")
    woutb_d = nc.dram_tensor("woutb", [H + 1, V], f32, kind="ExternalInput")
    woutbias_d = nc.dram_tensor("woutbias", [1, V], f32, kind="ExternalInput")
    iden_d = nc.dram_tensor("iden", [128, 128], f32, kind="ExternalInput")
    out_d = nc.dram_tensor("out", [V, BL], f32, kind="ExternalOutput")

    with tile.TileContext(nc) as tc:
        with (
            tc.tile_pool(name="const", bufs=1) as cp,
            tc.tile_pool(name="setup", bufs=1) as sp,
            tc.tile_pool(name="setup_ps", bufs=2, space="PSUM") as spp,
            tc.tile_pool(name="sweep", bufs=1) as swp,
            tc.tile_pool(name="qst_p", bufs=2) as qp,
            tc.tile_pool(name="vst", bufs=3) as vp,
            tc.tile_pool(name="ans_ps", bufs=1, space="PSUM") as ap_pool,
            tc.tile_pool(name="at_ps", bufs=2, space="PSUM") as atp,
        ):
            # ---- load constants ---------------------------------------
            def load(pool, dram, shape, tag, dtype=f32):
                t = pool.tile(shape, dtype, tag=tag, name=tag)
                nc.gpsimd.dma_start(out=t[:], in_=dram.ap())
                return t

            x0 = load(cp, x0_d, [BL, H], "c_x0")
            # first 8 steps' stream rows arrive as a plain contiguous DMA so
            # the chain starts without waiting for the first SWDGE gather
            qr0 = load(cp, qr0_d, [BL, 8 * 2 * H], "c_qr0")

            # ---- main sweep -------------------------------------------
            X = swp.tile([BL, H], f32, name="X")
            junk = swp.tile([BL, H], f32, name="junk")
            junkacc = swp.tile([BL, 1], f32, name="junkacc")
            alpha = swp.tile([BL, TP], f32, name="alpha")
            # only the padded tail column is read without being written
            nc.vector.memset(alpha[:, T:TP], 0.0)
            # one PSUM bank parks all 16x32 per-(chunk,b) answer columns,
            # b-major so the final reduce is a contiguous [H, BL, NCH] view
            cps_all = ap_pool.tile([H, BL, NCH], f32, name="cps_all")

            qtiles = {}
            vtiles = {}

            def issue_gathers(st, first=False):
                kix = qp.tile([128, SUPER * 8], i16, tag="kix", name="kix")
                nc.gpsimd.dma_start(
                    out=kix[:], in_=kidx_d.ap()[:, st * SUPER * 8:
                                                (st + 1) * SUPER * 8])
                q = qp.tile([128, SUPER, 2 * H], f32, tag="qst", name="qst")
                # HW SWDGE caps one gather at ~1024 idxs.  The first tile's
                # slots 0:8 come from the qr0 direct load instead (piece 0
                # skipped).
                npc = SUPER * 128 // 1024
                pieces = range(1 if first else 0, npc)
                for piece in pieces:
                    sl = SUPER // npc
                    nc.gpsimd.dma_gather(
                        out_ap=q[:, piece * sl:(piece + 1) * sl, :],
                        in_ap=qrtab_d.ap(),
                        idxs_ap=kix[:, piece * 64:(piece + 1) * 64],
                        num_idxs=1024, num_idxs_reg=1024, elem_size=2 * H)
                qtiles[st] = q

            def issue_vgathers(ci):
                vst = vp.tile([CHUNK, BL, H], f32, tag="vst", name="vst")
                vbase = ci * BL * CHUNK // 16
                for piece in range(BL * CHUNK // 1024):
                    nc.gpsimd.dma_gather(
                        out_ap=vst[:, piece * 8:(piece + 1) * 8, :],
                        in_ap=vtn_d.ap(),
                        idxs_ap=vidx_sb[:, vbase + piece * 64:
                                        vbase + (piece + 1) * 64],
                        num_idxs=1024, num_idxs_reg=1024, elem_size=H)
                vtiles[ci] = vst

            for st in range(min(1, NST)):
                issue_gathers(st, first=True)
            # bulk constants are not needed until the first answer chunk /
            # epilogue; load them after the first gathers are in flight
            vidx_sb = load(cp, vidx_d, [128, NCH * BL * CHUNK // 16],
                           "c_vidx", i16)
            wrpb = load(cp, wrpb_d, [H + 1, H], "c_wrpb")
            wrpbias = load(cp, wrpbias_d, [1, H], "c_wrpbias")
            woutb = load(cp, woutb_d, [H + 1, V], "c_woutb")
            woutbias = load(cp, woutbias_d, [1, V], "c_woutbias")
            iden = load(cp, iden_d, [128, 128], "c_iden")
            issue_vgathers(0)
            for st in range(NST):
                if st + 1 < NST:
                    issue_gathers(st + 1)
                    issue_vgathers(st + 1)
                q = qtiles.pop(st)
                t0 = st * SUPER
                sc = min(SUPER, T - t0)
                for j in range(sc):
                    tau = t0 + j
                    xin = x0 if tau == 0 else X
                    if tau < 8:
                        sQ = qr0[:, 2 * H * j:2 * H * j + H]
                        sR = qr0[:, 2 * H * j + H:2 * H * (j + 1)]
                    else:
                        sQ = q[:BL, j, 0:H]
                        sR = q[:BL, j, H:2 * H]
                    # accum = sum(X * (-k^2)) = -a_tau ; out is junk
                    nc.vector.affine_mul_reduce(
                        out=junk[:], accum_out=alpha[:, tau:tau + 1],
                        in0=xin[:], in1=sQ, scale=1.0, bias=0.0)
                    # X' = (X + (-a_tau)) * (k_tau / k_next)
                    nc.vector.affine_mul_reduce(
                        out=X[:], accum_out=junkacc[:],
                        in0=xin[:], in1=sR, scale=1.0,
                        bias=alpha[:, tau:tau + 1])
                # answer chunks of this supertile (full CHUNK frames; alpha
                # is zero-padded past T so junk v rows contribute nothing)
                for c0 in range(0, SUPER, CHUNK):
                    tau0 = t0 + c0
                    ci = tau0 // CHUNK
                    vst = vtiles.pop(ci)
                    at_ps = atp.tile([CHUNK, BL], f32, name="at_ps")
                    nc.tensor.transpose(at_ps[:],
                                        alpha[:, tau0:tau0 + CHUNK],
                                        iden[:BL, :BL])
                    atb = vp.tile([CHUNK, BL], f32, tag="atb", name="atb")
                    nc.scalar.copy(atb[:], at_ps[:])
                    for b in range(BL):
                        nc.tensor.matmul(cps_all[:, b, ci:ci + 1],
                                         lhsT=vst[:, b, :],
                                         rhs=atb[:, b:b + 1],
                                         start=True, stop=True)

            # ---- epilogue (output stays [V, BL]; host transposes) -----
            ones = sp.tile([1, BL], f32, name="ones")
            nc.vector.memset(ones[:], 1.0)
            ans_acc = sp.tile([H, BL], f32, name="ans_acc")
            nc.vector.tensor_reduce(
                ans_acc[:], cps_all[:, :, :],
                axis=mybir.AxisListType.X, op=Alu.add)
            rps = spp.tile([H, BL], f32, tag="sps", name="rps")
            nc.tensor.matmul(rps[:], lhsT=wrpbias[:], rhs=ones[:],
                             start=True, stop=False)
            nc.tensor.matmul(rps[:], lhsT=wrpb[:H, :], rhs=ans_acc[:],
                             start=False, stop=True)
            rx = sp.tile([H, BL], f32, name="rx")
            nc.scalar.copy(rx[:], rps[:])
            ops_ = spp.tile([V, BL], f32, tag="sps", name="ops_")
            nc.tensor.matmul(ops_[:], lhsT=woutbias[:], rhs=ones[:],
                             start=True, stop=False)
            nc.tensor.matmul(ops_[:], lhsT=woutb[:H, :], rhs=rx[:],
                             start=False, stop=True)
            o_sb = sp.tile([V, BL], f32, name="o_sb")
            nc.scalar.copy(o_sb[:], ops_[:])
            nc.gpsimd.dma_start(out=out_d.ap(), in_=o_sb[:])

    nc.compile()
    return nc


def _tables(inputs):
    """Host-side weight preprocessing: collapse the token-wise encode to
    64-row tables (pure function of the weights), mirroring reference.py."""
    f = np.float32
    embed = np.asarray(inputs["embed"], f)
    W1 = np.asarray(inputs["W1"], f)
    b1 = np.asarray(inputs["b1"], f)
    W2 = np.asarray(inputs["W2"], f)
    b2 = np.asarray(inputs["b2"], f)
    gamma = np.asarray(inputs["gamma"], f)
    beta = np.asarray(inputs["beta"], f)
    Wk = np.asarray(inputs["Wk"], f)
    Wv = np.asarray(inputs["Wv"], f)
    Wq = np.asarray(inputs["Wq"], f)

    e = embed  # [V, H]
    ff = np.maximum(e @ W1.T + b1, 0.0) @ W2.T + b2
    h = e + ff
    mu = h.mean(-1, keepdims=True)
    var = ((h - mu) ** 2).mean(-1, keepdims=True)
    hs = (h - mu) / np.sqrt(var + 1e-5) * gamma + beta
    k = hs @ Wk.T
    kn = k / np.maximum(np.linalg.norm(k, axis=-1, keepdims=True), 1e-12)
    vt = hs @ Wv.T
    qt = hs @ Wq.T
    return kn.astype(f), vt.astype(f), qt.astype(f)


def _marshal(inputs, T):
    f = np.float32
    seq = np.asarray(inputs["seq"])
    Wrp = np.asarray(inputs["Wrp"], f)
    brp = np.asarray(inputs["brp"], f)
    Wout = np.asarray(inputs["Wout"], f)
    bout = np.asarray(inputs["bout"], f)

    kn, vt, qt = _tables(inputs)
    # guard: the gauge divides by k components; clamp exact/near zeros
    # (|k| ~ 1e-12 changes the math by O(1e-24) but keeps ratios finite)
    kn = np.where(np.abs(kn) < 1e-12, 1e-12, kn).astype(f)

    # combined row for pair (a,b): [-k_a^2 | k_a / k_b]
    qrtab = np.empty((V, V, 2 * H), f)
    qrtab[:, :, :H] = (-kn * kn)[:, None, :]
    qrtab[:, :, H:] = kn[:, None, :] / kn[None, :, :]
    vtn = (-vt).astype(f)

    shared = {
        "qrtab": qrtab.reshape(V * V, 2 * H),
        "vtn": vtn,
        "wrpb": np.vstack([Wrp.T, brp[None]]).astype(f),
        "wrpbias": brp[None].astype(f),
        "woutb": np.vstack([Wout.T, bout[None]]).astype(f),
        "woutbias": bout[None].astype(f),
        "iden": np.eye(128, dtype=f),
    }
    TP = (T + SUPER - 1) // SUPER * SUPER
    NST = TP // SUPER
    NCH = TP // CHUNK

    def wrap(flat):
        n = flat.size
        w16 = np.ascontiguousarray(flat.reshape(n // 16, 16).T).astype(np.int16)
        return np.tile(w16, (8, 1))

    in_maps = []
    for c in range(NCORES):
        sl = slice(c * BL, (c + 1) * BL)
        sseq = seq[sl]
        # reversed-time ids: ids[b, tau] = seq[b, (T-1) - tau]
        ids = np.ascontiguousarray(sseq[:, T - 1::-1]).astype(np.int64)
        idsp = np.zeros((BL, TP), np.int64)
        idsp[:, :T] = ids
        # pair ids: cur*64 + next (next in tau order); tail pairs with self
        nxt = np.zeros((BL, TP), np.int64)
        nxt[:, :T - 1] = ids[:, 1:]
        nxt[:, T - 1:] = ids[:, T - 1:T]
        pair = idsp * 64 + nxt
        pair[:, T:] = 0
        # qr-stream: i = slot*128 + p ; p<BL -> pair[p, t0+slot], else 0
        kblocks = []
        for st in range(NST):
            blk = np.zeros((SUPER, 128), np.int64)
            blk[:, :BL] = pair[:, st * SUPER:(st + 1) * SUPER].T
            kblocks.append(wrap(blk.reshape(-1)))
        # v-stream: i = b*128 + tau ; chunk frames of CHUNK
        vblocks = []
        for ci in range(NCH):
            blk = idsp[:, ci * CHUNK:(ci + 1) * CHUNK]  # [BL, CHUNK]
            vblocks.append(wrap(blk.reshape(-1)))
        # X0 = q_row / k(first step) ; q uses the LAST token id seq[:, L-1]
        qrows = qt[sseq[:, L - 1]]
        krows = kn[ids[:, 0]]
        m = dict(shared)
        m["x0"] = (qrows / krows).astype(f)
        m["qr0"] = shared["qrtab"][pair[:, :8]].reshape(BL, 8 * 2 * H)
        m["kidx"] = np.concatenate(kblocks, axis=1)
        m["vidx"] = np.concatenate(vblocks, axis=1)
        in_maps.append(m)
    return in_maps


def kernel(**inputs):
    global LAST_RESULTS
    import os
    from concourse.bass_utils import run_bass_kernel_spmd

    T = T_FULL
    if "nc" not in _CACHE:
        _CACHE["nc"] = _build_nc(T)
    nc = _CACHE["nc"]
    in_maps = _marshal(inputs, T)
    trace = bool(int(os.environ.get("KERNEL_TRACE", "0")))
    res = run_bass_kernel_spmd(nc, in_maps, core_ids=list(range(NCORES)),
                               trace=trace)
    LAST_RESULTS = res
    out = np.concatenate([res.results[c]["out"].T for c in range(NCORES)],
                         axis=0)
    return out.astype(np.float32)
